# revision 5
# baseline (speedup 1.0000x reference)
"""Bahdanau-style additive attention kernel for Trainium2 (8 NeuronCores).

Computes, per batch b:
    q = query[b] @ W_q.T            # [F, H]
    c = context[b] @ W_c.T          # [S, H]
    E[f, s] = sum_h v[h] * tanh(q[f, h] + c[s, h])
    out[b] = softmax(E, axis=-1)    # [F, S]

Sharding: data-parallel over batch. 16 batches -> 8 cores x 2 batches.
Each core gets its own batch slice plus the full (tiny) W_q/W_c/v.
Inputs are pre-transposed on the host (queryT/contextT/W^T) so the
contraction dim lands on SBUF partitions without on-chip transposes.

Per-core dataflow (all shapes hardcoded):
  - PE projects to qT[h, f], cT[h, s] (h on partitions, 2 h-tiles).
  - DVE builds A[h, (ht, s, f)] = cT[h, s] + qT[h, f] with stride-0
    broadcast access patterns (one [128, 8192] instruction per s-block).
  - ACT applies tanh on the big tiles (fp16 output).
  - PE reduces over h against v: per s, matmul with the fp16 tanh tile
    as stationary [h=128, f=128] and v h-tile [128, 1] as moving,
    accumulating E[:, s] columns in PSUM as [f=128, s=256].
  - A tail fraction of s-values (BIAS_S) skips the DVE add and instead
    uses ACT's fused bias: tanh(qT + cT[:, s]) per (s, h-tile).
  - Softmax: DVE reduce_max(negate) -> ACT exp(E - max) with accum_out
    row-sum -> DVE reciprocal -> DVE scale -> DMA out.
"""

import sys

for _p in ("/opt/trn_rl_repo", "/opt/pypackages"):
    if _p not in sys.path:
        sys.path.append(_p)

from contextlib import ExitStack

import numpy as np

import concourse.bass as bass
import concourse.tile as tile
from concourse import mybir

B, F, S, D, H = 16, 128, 256, 256, 256
NCORES = 8
BPC = B // NCORES  # batches per core
S_BLK = 32         # s values per A/T working tile
BIAS_S = 0         # s values per batch routed through the ACT-bias path
T_DT = mybir.dt.float16  # dtype of tanh tiles + v (stationary path)
F32 = mybir.dt.float32
AF = mybir.ActivationFunctionType


def build_program(reps: int = 1) -> bass.Bass:
    nc = bass.Bass()
    qT_d = nc.declare_dram_parameter("queryT", [BPC, D, F], F32, isOutput=False)
    cT_d = nc.declare_dram_parameter("contextT", [BPC, D, S], F32, isOutput=False)
    wqT_d = nc.declare_dram_parameter("w_qT", [D, H], F32, isOutput=False)
    wcT_d = nc.declare_dram_parameter("w_cT", [D, H], F32, isOutput=False)
    v_d = nc.declare_dram_parameter("v", [H, 1], F32, isOutput=False)
    out_d = nc.declare_dram_parameter("out", [BPC, F, S], F32, isOutput=True)

    n_bias = BIAS_S
    n_blk = (S - n_bias) // S_BLK
    assert n_blk * S_BLK + n_bias == S

    with tile.TileContext(nc) as tc, ExitStack() as ctx:
        consts = ctx.enter_context(tc.tile_pool(name="consts", bufs=1))
        loads = ctx.enter_context(tc.tile_pool(name="loads", bufs=2))
        proj = ctx.enter_context(tc.tile_pool(name="proj", bufs=2))
        work = ctx.enter_context(tc.tile_pool(name="work", bufs=2))
        stats = ctx.enter_context(tc.tile_pool(name="stats", bufs=4))
        outp = ctx.enter_context(tc.tile_pool(name="outp", bufs=2))
        ps_scr = ctx.enter_context(tc.tile_pool(name="ps_scr", bufs=2, space="PSUM"))
        ps_e = ctx.enter_context(tc.tile_pool(name="ps_e", bufs=2, space="PSUM"))

        # v as two h-tiles: columns of a [128, 2] tile (cast to T_DT)
        v32 = consts.tile([128, 2], F32)
        for ht in range(2):
            nc.sync.dma_start(out=v32[:, ht : ht + 1], in_=v_d[128 * ht : 128 * (ht + 1), :])
        v_sb = consts.tile([128, 2], T_DT)
        nc.vector.tensor_copy(v_sb, v32)

        # W^T tiles: [d_part, d_chunk, h]
        wqT = consts.tile([128, 2, 256], F32)
        wcT = consts.tile([128, 2, 256], F32)
        for di in range(2):
            nc.sync.dma_start(out=wqT[:, di, :], in_=wqT_d[128 * di : 128 * (di + 1), :])
            nc.sync.dma_start(out=wcT[:, di, :], in_=wcT_d[128 * di : 128 * (di + 1), :])

        for rep in range(reps):
            for b in range(BPC):
                # ---- load pre-transposed query/context: [d_part, d_chunk, *] ----
                qryT = loads.tile([128, 2, 128], F32)
                ctxT = loads.tile([128, 2, 256], F32)
                for di in range(2):
                    nc.sync.dma_start(out=qryT[:, di, :], in_=qT_d[b, 128 * di : 128 * (di + 1), :])
                    nc.sync.dma_start(out=ctxT[:, di, :], in_=cT_d[b, 128 * di : 128 * (di + 1), :])

                # ---- projections: qT[h_part, ht, f], cT[h_part, ht, s] ----
                qT = proj.tile([128, 2, 128], F32)
                cT = proj.tile([128, 2, 256], F32)
                for ht in range(2):
                    qp = ps_scr.tile([128, 128], F32, tag="tp")
                    for di in range(2):
                        nc.tensor.matmul(
                            qp,
                            lhsT=wqT[:, di, 128 * ht : 128 * (ht + 1)],
                            rhs=qryT[:, di, :],
                            start=(di == 0),
                            stop=(di == 1),
                        )
                    nc.scalar.copy(out=qT[:, ht, :], in_=qp)
                    cp = ps_scr.tile([128, 256], F32, tag="cp")
                    for di in range(2):
                        nc.tensor.matmul(
                            cp,
                            lhsT=wcT[:, di, 128 * ht : 128 * (ht + 1)],
                            rhs=ctxT[:, di, :],
                            start=(di == 0),
                            stop=(di == 1),
                        )
                    nc.scalar.copy(out=cT[:, ht, :], in_=cp)

                # ---- main loop: E[f, s] accumulates in PSUM ----
                e_ps = ps_e.tile([128, 256], F32)
                for sb in range(n_blk):
                    s0 = sb * S_BLK
                    a_t = work.tile([128, 2, S_BLK, 128], F32)
                    nc.vector.tensor_add(
                        out=a_t,
                        in0=cT[:, :, s0 : s0 + S_BLK].unsqueeze(3).broadcast_to((128, 2, S_BLK, 128)),
                        in1=qT.unsqueeze(2).broadcast_to((128, 2, S_BLK, 128)),
                    )
                    t_t = work.tile([128, 2, S_BLK, 128], T_DT)
                    nc.scalar.activation(out=t_t, in_=a_t, func=AF.Tanh)
                    for sl in range(S_BLK):
                        s = s0 + sl
                        for ht in range(2):
                            nc.tensor.matmul(
                                e_ps[:, s : s + 1],
                                lhsT=t_t[:, ht, sl],
                                rhs=v_sb[:, ht : ht + 1],
                                start=(ht == 0),
                                stop=(ht == 1),
                            )
                # tail: ACT-bias route (add fused into tanh, small tiles)
                for s in range(S - n_bias, S):
                    t_b = work.tile([128, 2, 128], T_DT, tag="t_b")
                    for ht in range(2):
                        nc.scalar.activation(
                            out=t_b[:, ht],
                            in_=qT[:, ht, :],
                            func=AF.Tanh,
                            bias=cT[:, ht, s : s + 1],
                        )
                    for ht in range(2):
                        nc.tensor.matmul(
                            e_ps[:, s : s + 1],
                            lhsT=t_b[:, ht],
                            rhs=v_sb[:, ht : ht + 1],
                            start=(ht == 0),
                            stop=(ht == 1),
                        )

                # ---- softmax over s ----
                negmax = stats.tile([128, 1], F32)
                nc.vector.tensor_reduce(
                    out=negmax, in_=e_ps, axis=mybir.AxisListType.X,
                    op=mybir.AluOpType.max, negate=True,
                )
                p_sb = outp.tile([128, 256], F32)
                ssum = stats.tile([128, 1], F32)
                nc.scalar.activation(
                    out=p_sb, in_=e_ps, func=AF.Exp, bias=negmax, scale=1.0, accum_out=ssum,
                )
                rsum = stats.tile([128, 1], F32)
                nc.vector.reciprocal(rsum, ssum)
                nc.vector.tensor_scalar_mul(p_sb, in0=p_sb, scalar1=rsum)
                nc.sync.dma_start(out=out_d[b], in_=p_sb)

    # Walrus allows at most one semaphore wait per engine instruction; Tile
    # can attach several. Split them via event-semaphore joiners.
    import bass_rust

    bass_rust.generate_event_semaphores(nc)
    return nc


def host_prep(query, context, W_q, W_c, v):
    """Transpose inputs so the contraction dim is leading (per core slice)."""
    queryT = np.ascontiguousarray(np.transpose(query, (0, 2, 1)), dtype=np.float32)
    contextT = np.ascontiguousarray(np.transpose(context, (0, 2, 1)), dtype=np.float32)
    w_qT = np.ascontiguousarray(np.transpose(W_q), dtype=np.float32)
    w_cT = np.ascontiguousarray(np.transpose(W_c), dtype=np.float32)
    v2 = np.ascontiguousarray(v, dtype=np.float32).reshape(H, 1)
    return queryT, contextT, w_qT, w_cT, v2


_PROGRAM_CACHE: bass.Bass | None = None


def kernel(**inputs: np.ndarray) -> np.ndarray:
    global _PROGRAM_CACHE
    queryT, contextT, w_qT, w_cT, v2 = host_prep(
        inputs["query"], inputs["context"], inputs["W_q"], inputs["W_c"], inputs["v"]
    )

    if _PROGRAM_CACHE is None:
        _PROGRAM_CACHE = build_program()
    nc = _PROGRAM_CACHE

    in_maps = []
    for core in range(NCORES):
        b0 = core * BPC
        in_maps.append(
            {
                "queryT": queryT[b0 : b0 + BPC],
                "contextT": contextT[b0 : b0 + BPC],
                "w_qT": w_qT,
                "w_cT": w_cT,
                "v": v2,
            }
        )

    from concourse.bass_utils import run_bass_kernel_spmd

    res = run_bass_kernel_spmd(nc, in_maps, list(range(NCORES)))
    out = np.concatenate([res.results[i]["out"] for i in range(NCORES)], axis=0)
    return out.astype(np.float32)


if __name__ == "__main__":
    rng = np.random.default_rng(0)
    ins = {
        "query": rng.standard_normal((B, F, D), dtype=np.float32),
        "context": rng.standard_normal((B, S, D), dtype=np.float32),
        "W_q": rng.standard_normal((H, D), dtype=np.float32) / np.sqrt(D),
        "W_c": rng.standard_normal((H, D), dtype=np.float32) / np.sqrt(D),
        "v": rng.standard_normal((H,), dtype=np.float32),
    }
    o = kernel(**ins)
    print(o.shape, o.dtype, o.sum())


# revision 13
# speedup vs baseline: 7.2598x; 7.2598x over previous
"""Bahdanau-style additive attention kernel for Trainium2 (8 NeuronCores).

Computes, per batch b:
    q = query[b] @ W_q.T            # [F, H]
    c = context[b] @ W_c.T          # [S, H]
    E[f, s] = sum_h v[h] * tanh(q[f, h] + c[s, h])
    out[b] = softmax(E, axis=-1)    # [F, S]

Sharding: data-parallel over batch. 16 batches -> 8 cores x 2 batches.
Each core gets its own batch slice plus the full (tiny) W_q/W_c/v.
Inputs are pre-transposed on the host (queryT/contextT/W^T) so the
contraction dim lands on SBUF partitions without on-chip transposes.

Per-core dataflow (all shapes hardcoded):
  - PE projects to qT[h, f], cT[h, s] (h on partitions, 2 h-tiles).
  - DVE builds A[h, (ht, s, f)] = cT[h, s] + qT[h, f] with stride-0
    broadcast access patterns (one [128, 8192] instruction per s-block).
  - ACT applies tanh on the big tiles (fp16 output).
  - PE reduces over h against v: per s, matmul with the fp16 tanh tile
    as stationary [h=128, f=128] and v h-tile [128, 1] as moving,
    accumulating E[:, s] columns in PSUM as [f=128, s=256].
  - A tail fraction of s-values (BIAS_S) skips the DVE add and instead
    uses ACT's fused bias: tanh(qT + cT[:, s]) per (s, h-tile).
  - Softmax: DVE reduce_max(negate) -> ACT exp(E - max) with accum_out
    row-sum -> DVE reciprocal -> DVE scale -> DMA out.
"""

import sys

for _p in ("/opt/trn_rl_repo", "/opt/pypackages"):
    if _p not in sys.path:
        sys.path.append(_p)

from contextlib import ExitStack

import numpy as np

import concourse.bass as bass
import concourse.tile as tile
from concourse import mybir

B, F, S, D, H = 16, 128, 256, 256, 256
NCORES = 8
BPC = B // NCORES  # batches per core
S_BLK = 32         # s values per A/T working tile
BIAS_S = 0         # s values per batch routed through the ACT-bias path
ASSIST_BLK = 2     # s-blocks per batch whose adds run on PE (fp16 identity MMs)
T_DT = mybir.dt.float16  # dtype of tanh tiles + v (stationary path)
F16 = mybir.dt.float16
F32 = mybir.dt.float32
AF = mybir.ActivationFunctionType


def build_program(reps: int = 1) -> bass.Bass:
    nc = bass.Bass()
    qT_d = nc.declare_dram_parameter("queryT", [BPC, D, F], F32, isOutput=False)
    cT_d = nc.declare_dram_parameter("contextT", [BPC, D, S], F32, isOutput=False)
    wqT_d = nc.declare_dram_parameter("w_qT", [D, H], F32, isOutput=False)
    wcT_d = nc.declare_dram_parameter("w_cT", [D, H], F32, isOutput=False)
    v_d = nc.declare_dram_parameter("v", [H, 1], F32, isOutput=False)
    out_d = nc.declare_dram_parameter("out", [BPC, F, S], F32, isOutput=True)

    n_bias = BIAS_S
    n_blk = (S - n_bias) // S_BLK
    assert n_blk * S_BLK + n_bias == S

    with tile.TileContext(nc) as tc, ExitStack() as ctx:
        consts = ctx.enter_context(tc.tile_pool(name="consts", bufs=1))
        loads = ctx.enter_context(tc.tile_pool(name="loads", bufs=2))
        proj = ctx.enter_context(tc.tile_pool(name="proj", bufs=2))
        work = ctx.enter_context(tc.tile_pool(name="work", bufs=2))
        stats = ctx.enter_context(tc.tile_pool(name="stats", bufs=4))
        outp = ctx.enter_context(tc.tile_pool(name="outp", bufs=2))
        ps_scr = ctx.enter_context(tc.tile_pool(name="ps_scr", bufs=1, space="PSUM"))
        ps_e = ctx.enter_context(tc.tile_pool(name="ps_e", bufs=2, space="PSUM"))
        ps_a = ctx.enter_context(tc.tile_pool(name="ps_a", bufs=2, space="PSUM"))

        # v as two h-tiles: columns of a [128, 2] tile (cast to T_DT)
        v32 = consts.tile([128, 2], F32)
        for ht in range(2):
            nc.sync.dma_start(out=v32[:, ht : ht + 1], in_=v_d[128 * ht : 128 * (ht + 1), :])
        v_sb = consts.tile([128, 2], T_DT)
        nc.vector.tensor_copy(v_sb, v32)

        ident16 = None
        if ASSIST_BLK:
            from concourse.masks import make_identity

            ident16 = consts.tile([128, 128], F16)
            make_identity(nc, ident16)

        # W^T tiles: [d_part, d_chunk, h]
        wqT = consts.tile([128, 2, 256], F32)
        wcT = consts.tile([128, 2, 256], F32)
        for di in range(2):
            nc.sync.dma_start(out=wqT[:, di, :], in_=wqT_d[128 * di : 128 * (di + 1), :])
            nc.sync.dma_start(out=wcT[:, di, :], in_=wcT_d[128 * di : 128 * (di + 1), :])

        for rep in range(reps):
            for b in range(BPC):
                # ---- load pre-transposed query/context: [d_part, d_chunk, *] ----
                qryT = loads.tile([128, 2, 128], F32)
                ctxT = loads.tile([128, 2, 256], F32)
                for di in range(2):
                    nc.sync.dma_start(out=qryT[:, di, :], in_=qT_d[b, 128 * di : 128 * (di + 1), :])
                    nc.sync.dma_start(out=ctxT[:, di, :], in_=cT_d[b, 128 * di : 128 * (di + 1), :])

                # ---- projections: qT[h_part, ht, f], cT[h_part, ht, s] ----
                qT = proj.tile([128, 2, 128], F32)
                cT = proj.tile([128, 2, 256], F32)
                for ht in range(2):
                    qp = ps_scr.tile([128, 128], F32, tag="tp")
                    for di in range(2):
                        nc.tensor.matmul(
                            qp,
                            lhsT=wqT[:, di, 128 * ht : 128 * (ht + 1)],
                            rhs=qryT[:, di, :],
                            start=(di == 0),
                            stop=(di == 1),
                        )
                    nc.vector.tensor_copy(qT[:, ht, :], qp)
                    cp = ps_scr.tile([128, 256], F32, tag="cp")
                    for di in range(2):
                        nc.tensor.matmul(
                            cp,
                            lhsT=wcT[:, di, 128 * ht : 128 * (ht + 1)],
                            rhs=ctxT[:, di, :],
                            start=(di == 0),
                            stop=(di == 1),
                        )
                    nc.vector.tensor_copy(cT[:, ht, :], cp)

                # fp16 copies of the projections for the PE-assisted adds
                if ASSIST_BLK:
                    qT16 = proj.tile([128, 2, 128], F16)
                    cT16 = proj.tile([128, 2, 256], F16)
                    nc.vector.tensor_copy(qT16, qT)
                    nc.vector.tensor_copy(cT16, cT)

                # ---- main loop: E[f, s] accumulates in PSUM ----
                e_ps = ps_e.tile([128, 256], F32)
                for sb in range(n_blk):
                    s0 = sb * S_BLK
                    if sb < ASSIST_BLK:
                        # adds on PE: A[h, (s, f)] = I@q (bcast s) + I@c (bcast f)
                        t_t = work.tile([128, 2, S_BLK, 128], T_DT, tag="t_t")
                        for ht in range(2):
                            for sq in range(0, S_BLK, 8):
                                a_ps = ps_a.tile([128, 8, 128], F32, tag="a_ps")
                                for half in range(2):
                                    sl4 = slice(4 * half, 4 * half + 4)
                                    nc.tensor.matmul(
                                        a_ps[:, sl4],
                                        lhsT=ident16,
                                        rhs=qT16[:, ht].unsqueeze(1).broadcast_to((128, 4, 128)),
                                        start=True,
                                        stop=False,
                                    )
                                    nc.tensor.matmul(
                                        a_ps[:, sl4],
                                        lhsT=ident16,
                                        rhs=cT16[:, ht, s0 + sq + 4 * half : s0 + sq + 4 * half + 4]
                                        .unsqueeze(2)
                                        .broadcast_to((128, 4, 128)),
                                        start=False,
                                        stop=True,
                                    )
                                nc.scalar.activation(
                                    out=t_t[:, ht, sq : sq + 8], in_=a_ps, func=AF.Tanh
                                )
                    else:
                        a_t = work.tile([128, 2, S_BLK, 128], F32)
                        nc.vector.tensor_add(
                            out=a_t,
                            in0=cT[:, :, s0 : s0 + S_BLK].unsqueeze(3).broadcast_to((128, 2, S_BLK, 128)),
                            in1=qT.unsqueeze(2).broadcast_to((128, 2, S_BLK, 128)),
                        )
                        t_t = work.tile([128, 2, S_BLK, 128], T_DT, tag="t_t")
                        nc.scalar.activation(out=t_t, in_=a_t, func=AF.Tanh)
                    for sl in range(S_BLK):
                        s = s0 + sl
                        for ht in range(2):
                            nc.tensor.matmul(
                                e_ps[:, s : s + 1],
                                lhsT=t_t[:, ht, sl],
                                rhs=v_sb[:, ht : ht + 1],
                                start=(ht == 0),
                                stop=(ht == 1),
                            )
                # tail: ACT-bias route (add fused into tanh, small tiles)
                for s in range(S - n_bias, S):
                    t_b = work.tile([128, 2, 128], T_DT, tag="t_b")
                    for ht in range(2):
                        nc.scalar.activation(
                            out=t_b[:, ht],
                            in_=qT[:, ht, :],
                            func=AF.Tanh,
                            bias=cT[:, ht, s : s + 1],
                        )
                    for ht in range(2):
                        nc.tensor.matmul(
                            e_ps[:, s : s + 1],
                            lhsT=t_b[:, ht],
                            rhs=v_sb[:, ht : ht + 1],
                            start=(ht == 0),
                            stop=(ht == 1),
                        )

                # ---- softmax over s ----
                negmax = stats.tile([128, 1], F32)
                nc.vector.tensor_reduce(
                    out=negmax, in_=e_ps, axis=mybir.AxisListType.X,
                    op=mybir.AluOpType.max, negate=True,
                )
                p_sb = outp.tile([128, 256], F32)
                ssum = stats.tile([128, 1], F32)
                nc.scalar.activation(
                    out=p_sb, in_=e_ps, func=AF.Exp, bias=negmax, scale=1.0, accum_out=ssum,
                )
                rsum = stats.tile([128, 1], F32)
                nc.vector.reciprocal(rsum, ssum)
                nc.vector.tensor_scalar_mul(p_sb, in0=p_sb, scalar1=rsum)
                nc.sync.dma_start(out=out_d[b], in_=p_sb)

    # Walrus allows at most one semaphore wait per engine instruction; Tile
    # can attach several. Split them via event-semaphore joiners.
    import bass_rust

    bass_rust.generate_event_semaphores(nc)
    return nc


def host_prep(query, context, W_q, W_c, v):
    """Transpose inputs so the contraction dim is leading (per core slice)."""
    queryT = np.ascontiguousarray(np.transpose(query, (0, 2, 1)), dtype=np.float32)
    contextT = np.ascontiguousarray(np.transpose(context, (0, 2, 1)), dtype=np.float32)
    w_qT = np.ascontiguousarray(np.transpose(W_q), dtype=np.float32)
    w_cT = np.ascontiguousarray(np.transpose(W_c), dtype=np.float32)
    v2 = np.ascontiguousarray(v, dtype=np.float32).reshape(H, 1)
    return queryT, contextT, w_qT, w_cT, v2


_RUNNER_CACHE = None


def _make_runner():
    """Compile the program once; return f(concat_inputs) -> concat out."""
    import jax
    from jax.sharding import Mesh, PartitionSpec
    from jax.experimental.shard_map import shard_map
    from concourse import bass2jax

    nc = build_program()
    bass2jax.install_neuronx_cc_hook()
    partition_name = nc.partition_id_tensor.name if nc.partition_id_tensor else None
    in_names, out_names, out_avals = [], [], []
    for alloc in nc.m.functions[0].allocations:
        if not isinstance(alloc, mybir.MemoryLocationSet):
            continue
        name = alloc.memorylocations[0].name
        if alloc.kind == "ExternalInput":
            if name != partition_name:
                in_names.append(name)
        elif alloc.kind == "ExternalOutput":
            out_names.append(name)
            out_avals.append(
                jax.core.ShapedArray(tuple(alloc.tensor_shape), mybir.dt.np(alloc.dtype))
            )
    n_params = len(in_names)
    all_in_names = list(in_names) + out_names
    if partition_name is not None:
        all_in_names.append(partition_name)

    def _body(*args):
        operands = list(args)
        if partition_name is not None:
            operands.append(bass2jax.partition_id_tensor())
        return tuple(
            bass2jax._bass_exec_p.bind(
                *operands,
                out_avals=tuple(out_avals),
                in_names=tuple(all_in_names),
                out_names=tuple(out_names),
                lowering_input_output_aliases=(),
                sim_require_finite=True,
                sim_require_nnan=True,
                nc=nc,
            )
        )

    devices = jax.devices()[:NCORES]
    mesh = Mesh(np.asarray(devices), ("core",))
    n_outs = len(out_names)
    sharded = jax.jit(
        shard_map(
            _body,
            mesh=mesh,
            in_specs=(PartitionSpec("core"),) * (n_params + n_outs),
            out_specs=(PartitionSpec("core"),) * n_outs,
            check_rep=False,
        ),
        keep_unused=True,
    )
    zeros = [np.zeros((NCORES * a.shape[0], *a.shape[1:]), a.dtype) for a in out_avals]
    oi = out_names.index("out")

    def run(by_name: dict):
        args = [by_name[n] for n in in_names] + zeros
        out = sharded(*args)
        return np.asarray(out[oi])

    return run


def kernel(**inputs: np.ndarray) -> np.ndarray:
    global _RUNNER_CACHE
    queryT, contextT, w_qT, w_cT, v2 = host_prep(
        inputs["query"], inputs["context"], inputs["W_q"], inputs["W_c"], inputs["v"]
    )
    if _RUNNER_CACHE is None:
        _RUNNER_CACHE = _make_runner()
    out = _RUNNER_CACHE(
        {
            "queryT": queryT.reshape(B, D, F),
            "contextT": contextT.reshape(B, D, S),
            "w_qT": np.broadcast_to(w_qT, (NCORES, D, H)).reshape(NCORES * D, H),
            "w_cT": np.broadcast_to(w_cT, (NCORES, D, H)).reshape(NCORES * D, H),
            "v": np.broadcast_to(v2, (NCORES, H, 1)).reshape(NCORES * H, 1),
        }
    )
    return np.ascontiguousarray(out.reshape(B, F, S).astype(np.float32))


if __name__ == "__main__":
    rng = np.random.default_rng(0)
    ins = {
        "query": rng.standard_normal((B, F, D), dtype=np.float32),
        "context": rng.standard_normal((B, S, D), dtype=np.float32),
        "W_q": rng.standard_normal((H, D), dtype=np.float32) / np.sqrt(D),
        "W_c": rng.standard_normal((H, D), dtype=np.float32) / np.sqrt(D),
        "v": rng.standard_normal((H,), dtype=np.float32),
    }
    o = kernel(**ins)
    print(o.shape, o.dtype, o.sum())
